# revision 2
# baseline (speedup 1.0000x reference)
"""GPT-2 (no-softmax attention) dense transformer on 8 TRN2 NeuronCores.

v3 = v2 (fp8 DoubleRow GEMMs, factorized attention, pair-AllReduce of
S) + overlap fixes measured from the v2 trace:
  - k/v GEMMs run c_out-half-major; S Gram blocks + the AllReduce for
    each half launch as soon as that half is done, so collective #1
    starts at the layer midpoint and both hide under k/v-half2 + q.
  - y drains alternate scalar/vector and py pool has 4 bufs (the v2 y
    stage was gated by one engine's drain rate through 2 bufs).
  - next layer's x -> fp8 casts are interleaved into the proj drain
    stream so the k/v GEMMs of layer l+1 start immediately.

Scales (empirical absmax in parens; TRN fp8 e4m3 max normal 240,
overflow -> inf on HW while ml_dtypes saturates in the sim):
  weights fp8 = W * 2^12            (~5)
  x8          = x                   (~5)
  kn/vn       = k * 2^11            (~80)
  S psum      = S_own * 2^22 -> s_sb bf16 = S_own * 2^15 (AllReduce)
  s8          = S * 2^11            (~47, block-diag per head pair)
  qt          = (q/8) * 2^12        (~18)
  y psum      = y * 2^23 -> yt fp8 = y * 2^16 (~8)
  proj psum   = p * 2^28 -> xT += psum * 2^-28 (fp32 residual, fused)
"""

import sys

if "/opt/trn_rl_repo" not in sys.path:
    sys.path.insert(0, "/opt/trn_rl_repo")

import numpy as np

N_LAYER = 12
N_EMBD = 1024
T_OWN = 1024
D = 64

_CACHE = {}

SW = 4096.0          # 2^12 host weight scale


def build(L, C, T_own, zero_bias):
    import concourse.bacc as bacc
    import concourse.mybir as mybir
    from concourse import tile

    f32 = mybir.dt.float32
    bf16 = mybir.dt.bfloat16
    f8 = mybir.dt.float8e4
    DR = mybir.MatmulPerfMode.DoubleRow
    Copy = mybir.ActivationFunctionType.Copy

    H = C // D
    NCT = C // 128               # 128-wide c chunks
    NG = NCT // 2                # DoubleRow chunk pairs
    NTT = T_own // 128           # 128-wide own-t chunks
    NTH = T_own // 512           # 512-wide t slices
    HP = H // 2                  # head pairs
    HH = HP // 2                 # head pairs per c_out half
    CW = 512
    groups = [[0, 1], [2, 3], [4, 5], [6, 7]]

    nc = bacc.Bacc("TRN2", target_bir_lowering=False, debug=False, num_devices=8)

    xT_in = nc.dram_tensor("xT", [NCT, 128, T_own], f32, kind="ExternalInput")
    wq_in = nc.dram_tensor("wq", [L, NG, 128, 2 * C], f8, kind="ExternalInput")
    wk_in = nc.dram_tensor("wk", [L, NG, 128, 2 * C], f8, kind="ExternalInput")
    wv_in = nc.dram_tensor("wv", [L, NG, 128, 2 * C], f8, kind="ExternalInput")
    wp_in = nc.dram_tensor("wp", [L, NG, 128, 2 * C], f8, kind="ExternalInput")
    if not zero_bias:
        bq_in = nc.dram_tensor("bq", [L, 128, NCT], f32, kind="ExternalInput")
        bkv_in = nc.dram_tensor("bkv", [L, 1, 2 * C], bf16, kind="ExternalInput")
        bp_in = nc.dram_tensor("bp", [L, 128, NCT], f32, kind="ExternalInput")
    out_xT = nc.dram_tensor("out", [NCT, 128, T_own], f32, kind="ExternalOutput")

    with tile.TileContext(nc) as tc:
        with (
            tc.tile_pool(name="persist", bufs=1) as persist,
            tc.tile_pool(name="dram", bufs=1, space="DRAM") as dram,
            tc.tile_pool(name="wpool", bufs=6) as wpool,
            tc.tile_pool(name="bias", bufs=2) as bias_pool,
            tc.tile_pool(name="pm", bufs=3, space="PSUM") as pm,
            tc.tile_pool(name="psm", bufs=1, space="PSUM") as psm,
            tc.tile_pool(name="py", bufs=4, space="PSUM") as py,
        ):
            xT = persist.tile([128, NCT, T_own], f32)
            x8 = persist.tile([128, NCT, T_own], f8)
            qt = persist.tile([128, NCT, T_own], f8)
            kn = persist.tile([128, NTT, C], f8)
            vn = persist.tile([128, NTT, C], f8)
            yt = persist.tile([128, NCT, T_own], f8)
            s_sb = persist.tile([128, HP, 64], bf16)
            s_rbf = persist.tile([128, HP, 64], bf16)
            s8 = persist.tile([128, HP, 128], f8)
            nc.gpsimd.memset(s8[:], 0.0)
            if not zero_bias:
                ones_b = persist.tile([1, 128], bf16)
                nc.gpsimd.memset(ones_b[:], 1.0)

            s_send = [dram.tile([128, HH, 64], bf16, name=f"ssend{h}")
                      for h in range(2)]
            s_recv = [dram.tile([128, HH, 64], bf16, name=f"srecv{h}")
                      for h in range(2)]

            for ci in range(NCT):
                nc.sync.dma_start(xT[:, ci, :], xT_in[ci])

            for l in range(L):
                # ---- weight loads for k, v, q (sync DMA queue)
                wk_t = wpool.tile([128, NCT, C], f8, tag="w", name=f"wk{l}")
                for g in range(NG):
                    nc.sync.dma_start(wk_t[:, 2 * g : 2 * g + 2, :], wk_in[l, g])
                wv_t = wpool.tile([128, NCT, C], f8, tag="w", name=f"wv{l}")
                for g in range(NG):
                    nc.sync.dma_start(wv_t[:, 2 * g : 2 * g + 2, :], wv_in[l, g])
                wq_t = wpool.tile([128, NCT, C], f8, tag="w", name=f"wq{l}")
                for g in range(NG):
                    nc.sync.dma_start(wq_t[:, 2 * g : 2 * g + 2, :], wq_in[l, g])
                if not zero_bias:
                    bkv_t = bias_pool.tile([1, 2 * C], bf16, tag="bkv")
                    nc.sync.dma_start(bkv_t[:], bkv_in[l])
                    bq_t = bias_pool.tile([128, NCT], f32, tag="bq")
                    nc.sync.dma_start(bq_t[:], bq_in[l])
                    bp_t = bias_pool.tile([128, NCT], f32, tag="bp")
                    nc.sync.dma_start(bp_t[:], bp_in[l])

                if l == 0:
                    # first layer: cast everything up front
                    for g in range(NG):
                        if g % 2 == 0:
                            nc.vector.tensor_copy(
                                x8[:, 2 * g : 2 * g + 2, :],
                                xT[:, 2 * g : 2 * g + 2, :],
                            )
                        else:
                            nc.scalar.activation(
                                x8[:, 2 * g : 2 * g + 2, :],
                                xT[:, 2 * g : 2 * g + 2, :],
                                Copy,
                            )

                # ---- k, v GEMMs c_out-half-major; S + AllReduce per half
                for ch in range(2):
                    for w_t, base, dest in ((wk_t, 0, kn), (wv_t, C, vn)):
                        for tt in range(NTT):
                            ps = pm.tile([128, CW], f32, tag="pm")
                            for g in range(NG):
                                nc.tensor.matmul(
                                    ps[:],
                                    x8[:, 2 * g : 2 * g + 2, tt * 128 : (tt + 1) * 128],
                                    w_t[:, 2 * g : 2 * g + 2, ch * CW : (ch + 1) * CW],
                                    start=(g == 0),
                                    stop=(g == NG - 1) and zero_bias,
                                    perf_mode=DR,
                                )
                            if not zero_bias:
                                nc.tensor.matmul(
                                    ps[:],
                                    ones_b[:, 0:128],
                                    bkv_t[:, base + ch * CW : base + (ch + 1) * CW],
                                    start=False,
                                    stop=True,
                                )
                            # kn/vn = k * 2^11 (psum holds k * 2^12)
                            if ch == 0:
                                nc.scalar.activation(
                                    dest[:, tt, ch * CW : (ch + 1) * CW],
                                    ps[:],
                                    Copy,
                                    scale=0.5,
                                )
                            else:
                                nc.vector.tensor_scalar_mul(
                                    dest[:, tt, ch * CW : (ch + 1) * CW], ps[:], 0.5
                                )

                    # S Gram blocks for this half's head pairs
                    for j in range(HH * ch, HH * ch + HH):
                        sp = psm.tile([128, 128], f32, tag="ps")
                        for tg in range(NTT // 2):
                            nc.tensor.matmul(
                                sp[:],
                                kn[:, 2 * tg : 2 * tg + 2, j * 128 : (j + 1) * 128],
                                vn[:, 2 * tg : 2 * tg + 2, j * 128 : (j + 1) * 128],
                                start=(tg == 0),
                                stop=(tg == NTT // 2 - 1),
                                perf_mode=DR,
                            )
                        # diagonal blocks; psum = S_own * 2^22 -> bf16 S_own * 2^15
                        nc.vector.tensor_scalar_mul(
                            s_sb[0:64, j, :], sp[0:64, 0:64], 1.0 / 128.0
                        )
                        nc.vector.tensor_scalar_mul(
                            s_sb[64:128, j, :], sp[64:128, 64:128], 1.0 / 128.0
                        )
                    nc.sync.dma_start(
                        s_send[ch][:], s_sb[:, HH * ch : HH * ch + HH, :]
                    )
                    nc.gpsimd.collective_compute(
                        "AllReduce",
                        mybir.AluOpType.add,
                        replica_groups=groups,
                        ins=[s_send[ch].opt()],
                        outs=[s_recv[ch].opt()],
                    )

                # ---- q tiles (transposed layout), overlap the AllReduces
                for co in range(NCT):
                    for th in range(NTH):
                        ps = pm.tile([128, CW], f32, tag="pm")
                        for g in range(NG):
                            nc.tensor.matmul(
                                ps[:],
                                wq_t[:, 2 * g : 2 * g + 2, co * 128 : (co + 1) * 128],
                                x8[:, 2 * g : 2 * g + 2, th * CW : (th + 1) * CW],
                                start=(g == 0),
                                stop=(g == NG - 1),
                                perf_mode=DR,
                            )
                        if zero_bias:
                            nc.vector.tensor_scalar_mul(
                                qt[:, co, th * CW : (th + 1) * CW], ps[:], 0.125
                            )
                        else:
                            nc.vector.tensor_scalar(
                                qt[:, co, th * CW : (th + 1) * CW],
                                ps[:],
                                bq_t[:, co : co + 1],
                                0.125,
                                op0=mybir.AluOpType.add,
                                op1=mybir.AluOpType.mult,
                            )

                # ---- S halves back; block-diagonal fp8 S on scalar engine
                for ch in range(2):
                    lo, hi = HH * ch, HH * ch + HH
                    nc.sync.dma_start(s_rbf[:, lo:hi, :], s_recv[ch][:])
                    nc.scalar.activation(
                        s8[0:64, lo:hi, 0:64], s_rbf[0:64, lo:hi, :],
                        Copy, scale=0.0625,
                    )
                    nc.scalar.activation(
                        s8[64:128, lo:hi, 64:128], s_rbf[64:128, lo:hi, :],
                        Copy, scale=0.0625,
                    )

                # ---- yT per head pair; drains alternate scalar/vector
                for j in range(HP):
                    for th in range(NTH):
                        yp = py.tile([128, CW], f32, tag="py")
                        nc.tensor.matmul(
                            yp[:],
                            s8[:, j, :],
                            qt[:, j, th * CW : (th + 1) * CW],
                            start=True,
                            stop=True,
                        )
                        if j % 2 == 0:
                            nc.scalar.activation(
                                yt[:, j, th * CW : (th + 1) * CW],
                                yp[:],
                                Copy,
                                scale=1.0 / 128.0,
                            )
                        else:
                            nc.vector.tensor_scalar_mul(
                                yt[:, j, th * CW : (th + 1) * CW],
                                yp[:],
                                1.0 / 128.0,
                            )

                # ---- proj + residual; interleave next layer's x8 casts
                wp_t = wpool.tile([128, NCT, C], f8, tag="w", name=f"wp{l}")
                for g in range(NG):
                    nc.sync.dma_start(wp_t[:, 2 * g : 2 * g + 2, :], wp_in[l, g])
                for co in range(NCT):
                    for th in range(NTH):
                        ps = pm.tile([128, CW], f32, tag="pm")
                        for g in range(NG):
                            nc.tensor.matmul(
                                ps[:],
                                wp_t[:, 2 * g : 2 * g + 2, co * 128 : (co + 1) * 128],
                                yt[:, 2 * g : 2 * g + 2, th * CW : (th + 1) * CW],
                                start=(g == 0),
                                stop=(g == NG - 1),
                                perf_mode=DR,
                            )
                        if not zero_bias:
                            nc.vector.tensor_scalar_add(
                                ps[:], ps[:], bp_t[:, co : co + 1]
                            )
                        nc.vector.scalar_tensor_tensor(
                            xT[:, co, th * CW : (th + 1) * CW],
                            ps[:],
                            1.0 / float(2**28),
                            xT[:, co, th * CW : (th + 1) * CW],
                            op0=mybir.AluOpType.mult,
                            op1=mybir.AluOpType.add,
                        )
                    if co % 2 == 1 and l < L - 1:
                        g = co // 2
                        if g % 2 == 0:
                            nc.vector.tensor_copy(
                                x8[:, 2 * g : 2 * g + 2, :],
                                xT[:, 2 * g : 2 * g + 2, :],
                            )
                        else:
                            nc.scalar.activation(
                                x8[:, 2 * g : 2 * g + 2, :],
                                xT[:, 2 * g : 2 * g + 2, :],
                                Copy,
                            )

            for ci in range(NCT):
                nc.sync.dma_start(out_xT[ci], xT[:, ci, :])

    nc.compile()
    return nc


def pack_inputs(inputs_embeds, Wqkv, bqkv, Wproj, bproj, L, C, T_own, zero_bias):
    """Host-side shard + relayout. Returns in_maps for the 8 cores."""
    import ml_dtypes

    f8 = ml_dtypes.float8_e4m3
    bf16 = ml_dtypes.bfloat16
    NCT = C // 128
    NG = NCT // 2

    def pack_w(W):  # [L, C_out, C_in] -> [L, NG, 128, 2C] fp8 * 2^12
        Wt = W.transpose(0, 2, 1).reshape(L, NG, 2, 128, C)
        Wt = np.ascontiguousarray(Wt.transpose(0, 1, 3, 2, 4)).reshape(
            L, NG, 128, 2 * C
        )
        return (Wt * SW).astype(f8)

    wq = pack_w(Wqkv[:, :C, :])
    wk = pack_w(Wqkv[:, C : 2 * C, :])
    wv = pack_w(Wqkv[:, 2 * C : 3 * C, :])
    wp = pack_w(Wproj)

    base = {"wq": wq, "wk": wk, "wv": wv, "wp": wp}
    if not zero_bias:
        base["bq"] = np.ascontiguousarray(
            (bqkv[:, :C] * SW).reshape(L, NCT, 128).transpose(0, 2, 1)
        ).astype(np.float32)
        base["bkv"] = (bqkv[:, C : 3 * C] * SW).reshape(L, 1, 2 * C).astype(bf16)
        base["bp"] = np.ascontiguousarray(
            (bproj * float(2**28)).reshape(L, NCT, 128).transpose(0, 2, 1)
        ).astype(np.float32)

    in_maps = []
    for core in range(8):
        b, s = core // 2, core % 2
        xs = inputs_embeds[b, s * T_own : (s + 1) * T_own, :]  # [T_own, C]
        xT = np.ascontiguousarray(xs.T).reshape(NCT, 128, T_own).astype(np.float32)
        m = {"xT": xT}
        m.update(base)
        in_maps.append(m)
    return in_maps


def run_model(inputs_embeds, Wqkv, bqkv, Wproj, bproj, L, C, T_own, trace=False,
              tmpdir=None):
    from concourse.bass_utils import run_bass_kernel_spmd

    zero_bias = not (np.any(bqkv) or np.any(bproj))
    key = (L, C, T_own, zero_bias)
    if key not in _CACHE:
        _CACHE[key] = build(L, C, T_own, zero_bias)
    nc = _CACHE[key]
    in_maps = pack_inputs(
        inputs_embeds, Wqkv, bqkv, Wproj, bproj, L, C, T_own, zero_bias
    )
    res = run_bass_kernel_spmd(
        nc, in_maps, core_ids=list(range(8)), trace=trace, tmpdir=tmpdir
    )
    Bfull, T = inputs_embeds.shape[0], inputs_embeds.shape[1]
    out = np.empty((Bfull, T, C), dtype=np.float32)
    for core in range(8):
        b, s = core // 2, core % 2
        o = res.results[core]["out"].reshape(C, T_own)
        out[b, s * T_own : (s + 1) * T_own, :] = o.T
    return out, res


def kernel(**inputs):
    out, _ = run_model(
        inputs["inputs_embeds"],
        inputs["Wqkv"],
        inputs["bqkv"],
        inputs["Wproj"],
        inputs["bproj"],
        N_LAYER,
        N_EMBD,
        T_OWN,
    )
    return out
